# revision 2
# baseline (speedup 1.0000x reference)
"""Bass/Trainium2 kernel for a 2-layer GCN (GCNConv -> ReLU -> GCNConv ->
ReLU -> global_mean_pool -> Linear), distributed over 8 NeuronCores.

v2 strategy (graph/data parallel): nodes are split into 391 blocks of 128;
blocks are LPT-assigned to the 8 cores to balance edge counts, and sorted
by edge count within each core so the per-position cross-core max padding
is minimal.  The host pre-gathers each core's per-edge source rows into a
dense fp8 stream in SBUF layout (sequential DMA on device - no SWDGE
gather), and precomputes the per-edge selection matrices M (fp8, with the
GCN norm scaled x16 folded in; weights are scaled 1/16 to compensate).
On device, each 128-dst-node position does:
  zT[k] = sum_j stream[:, j, 128k:128k+128].T @ M[:, j, :]   (fp8 DoubleRow)
  h     = sum_k zT[k].T @ W[k] + onehot.T @ biasrow          (bf16, PSUM)
  h     = relu(h) -> fp8                                      (Act engine)
Layer 1 writes h1 (fp8) per position; the host reassembles h1, pre-gathers
the layer-2 stream, and launch 2 repeats the same pipeline, additionally
accumulating per-graph pooling sums via one extra matmul per position.
The final (tiny) mean + [512,1] linear runs on the host.
"""
import sys
sys.path.insert(0, "/opt/trn_rl_repo")

import numpy as np
import ml_dtypes
from contextlib import ExitStack

from concourse import mybir
import concourse.bacc as bacc
import concourse.tile as tile
from concourse.bass_utils import run_bass_kernel_spmd

P = 128
N_NODES = 50000
N_EDGES = 800000
IN_CH = 256
HID = 512
N_GRAPHS = 64
NCORES = 8
NGB = (N_NODES + P - 1) // P       # 391 global blocks of 128 nodes
NPOS = (NGB + NCORES - 1) // NCORES  # 49 positions per core
GRP = 6                            # positions per DMA group

F32 = mybir.dt.float32
BF16 = mybir.dt.bfloat16
F8 = mybir.dt.float8e4
NP_F8 = ml_dtypes.float8_e4m3
NP_BF16 = ml_dtypes.bfloat16

MSCALE = np.float32(16.0)          # norm scale folded into M; W scaled 1/16


def _build_layer(F_in, F_out, Cb_list, W, bases, layer2):
    """Build + compile the bass module for one GCN layer (SPMD, per-core).

    W: width of the compact selection matrices; bases[p][q] is the static
    dst-column base of pair q at position p (shared across cores)."""
    TOTC = sum(Cb_list)
    KT = F_in // P
    coffs = np.concatenate([[0], np.cumsum(Cb_list)]).astype(int)
    # ramped position-group sizes: small first groups shorten pipeline fill,
    # small last groups shorten the compute drain after the final DMA
    head, tail = [1, 1, 2], [2, 1, 1]
    sizes = list(head)
    while sum(sizes) + GRP <= NPOS - sum(tail):
        sizes.append(GRP)
    mid_rem = NPOS - sum(sizes) - sum(tail)
    if mid_rem > 0:
        sizes.append(mid_rem)
    sizes += tail
    gstart = np.concatenate([[0], np.cumsum(sizes)]).astype(int)
    grp_of = np.searchsorted(gstart, np.arange(NPOS), side="right") - 1
    goff = [coffs[gstart[g]] for g in range(len(sizes))]
    gsz = [coffs[gstart[g + 1]] - goff[g] for g in range(len(sizes))]

    nc = bacc.Bacc("TRN2", target_bir_lowering=False, debug=False)
    # stream rows and selection matrices are interleaved per chunk:
    # sm[:, j, :F_in] = gathered rows, sm[:, j, F_in:] = compact M columns
    sm_d = nc.dram_tensor("sm", [P, TOTC, F_in + W], F8, kind="ExternalInput")
    w_d = nc.dram_tensor("w", [P, KT, F_out], BF16, kind="ExternalInput")
    biasrow_d = nc.dram_tensor("biasrow", [P, F_out], BF16, kind="ExternalInput")
    onehot_d = nc.dram_tensor("onehot", [P, P], BF16, kind="ExternalInput")
    if layer2:
        batchloc_d = nc.dram_tensor("batchloc", [P, NPOS], BF16, kind="ExternalInput")
        iota64_d = nc.dram_tensor("iota64", [P, 1, N_GRAPHS], BF16, kind="ExternalInput")
        pout_d = nc.dram_tensor("pout", [N_GRAPHS, F_out], F32, kind="ExternalOutput")
    else:
        hout_d = nc.dram_tensor("hout", [P, NPOS, F_out], F8, kind="ExternalOutput")

    with tile.TileContext(nc) as tc, ExitStack() as ctx:
        const = ctx.enter_context(tc.tile_pool(name="const", bufs=1))
        sg = ctx.enter_context(tc.tile_pool(name="sg", bufs=3))
        hst = ctx.enter_context(tc.tile_pool(name="hst", bufs=2))
        work = ctx.enter_context(tc.tile_pool(name="work", bufs=3))
        zps = ctx.enter_context(tc.tile_pool(name="zps", bufs=2, space="PSUM"))
        hps = ctx.enter_context(tc.tile_pool(name="hps", bufs=2, space="PSUM"))
        if layer2:
            pps = ctx.enter_context(tc.tile_pool(name="pps", bufs=1, space="PSUM"))

        GMAX = max(gsz)
        sg_t = hst_t = None
        # first stream group goes out before the consts so compute can start
        sg_t = sg.tile([P, GMAX, F_in + W], F8, tag="sg")
        nc.sync.dma_start(sg_t[:, :gsz[0], :], sm_d[:, 0:gsz[0], :])

        w_sb = const.tile([P, KT, F_out], BF16)
        nc.sync.dma_start(w_sb[:], w_d[:])
        biasrow_sb = const.tile([P, F_out], BF16)
        nc.sync.dma_start(biasrow_sb[:], biasrow_d[:])
        onehot_sb = const.tile([P, P], BF16)
        nc.sync.dma_start(onehot_sb[:], onehot_d[:])
        czero = const.tile([P, 2, KT * P], F8)
        nc.vector.memset(czero[:], 0.0)
        if layer2:
            batchloc_sb = const.tile([P, NPOS], BF16)
            nc.sync.dma_start(batchloc_sb[:], batchloc_d[:])
            iota64_sb = const.tile([P, 1, N_GRAPHS], BF16)
            nc.sync.dma_start(iota64_sb[:], iota64_d[:])
            pool_ps = pps.tile([N_GRAPHS, F_out], F32)

        for p in range(NPOS):
            g, Cb = int(grp_of[p]), Cb_list[p]
            if p == gstart[g]:
                if p > 0:
                    sg_t = sg.tile([P, GMAX, F_in + W], F8, tag="sg")
                    nc.sync.dma_start(sg_t[:, :gsz[g], :],
                                      sm_d[:, goff[g]:goff[g] + gsz[g], :])
                if not layer2:
                    hst_t = hst.tile([P, max(sizes), F_out], F8, tag="hst")
            o = coffs[p] - goff[g]

            zT_ps = zps.tile([P, KT, P], F32)
            # zero the whole bank with one DoubleRow matmul of zeros
            nc.tensor.matmul(
                zT_ps[:], czero[:, :, :P], czero[:],
                perf_mode=mybir.MatmulPerfMode.DoubleRow,
                start=True, stop=False, skip_group_check=True)
            nlast = (KT - 1, len(bases[p]) - 1)
            for k in range(KT):
                for q, base in enumerate(bases[p]):
                    j = 2 * q
                    stop = (k, q) == nlast
                    if j + 1 < Cb:
                        nc.tensor.matmul(
                            zT_ps[:, k, base:base + W],
                            sg_t[:, o + j:o + j + 2, k * P:(k + 1) * P],
                            sg_t[:, o + j:o + j + 2, F_in:F_in + W],
                            perf_mode=mybir.MatmulPerfMode.DoubleRow,
                            start=False, stop=stop, skip_group_check=True)
                    else:
                        nc.tensor.matmul(
                            zT_ps[:, k, base:base + W],
                            sg_t[:, o + j, k * P:(k + 1) * P],
                            sg_t[:, o + j, F_in:F_in + W],
                            start=False, stop=stop, skip_group_check=True)

            zT_sb = work.tile([P, KT, P], BF16, tag="zT")
            nc.vector.tensor_copy(zT_sb[:], zT_ps[:])

            h_ps = hps.tile([P, F_out], F32)
            for k in range(KT):
                nc.tensor.matmul(h_ps[:], zT_sb[:, k, :], w_sb[:, k, :],
                                 start=(k == 0), stop=False)
            nc.tensor.matmul(h_ps[:], onehot_sb[:], biasrow_sb[:],
                             start=False, stop=True)

            if layer2:
                h_sb = work.tile([P, F_out], BF16, tag="h")
                nc.scalar.activation(h_sb[:], h_ps[:],
                                     mybir.ActivationFunctionType.Relu)
                G = work.tile([P, 1, N_GRAPHS], BF16, tag="G")
                nc.vector.tensor_tensor(
                    out=G[:],
                    in0=batchloc_sb[:, p:p + 1].to_broadcast([P, 1, N_GRAPHS]),
                    in1=iota64_sb[:],
                    op=mybir.AluOpType.is_equal)
                nc.tensor.matmul(pool_ps[:], G[:, 0, :], h_sb[:],
                                 start=(p == 0), stop=(p == NPOS - 1),
                                 skip_group_check=True)
            else:
                nc.scalar.activation(hst_t[:, p - gstart[g], :], h_ps[:],
                                     mybir.ActivationFunctionType.Relu)
                if p == gstart[g + 1] - 1:
                    gcnt = p - gstart[g] + 1
                    nc.sync.dma_start(
                        hout_d[:, gstart[g]:gstart[g] + gcnt, :],
                        hst_t[:, :gcnt, :])

        if layer2:
            p_sb = work.tile([N_GRAPHS, F_out], F32, tag="p")
            nc.vector.tensor_copy(p_sb[:], pool_ps[:])
            nc.sync.dma_start(pout_d[:, :], p_sb[:])

    nc.compile()
    return nc


def _preprocess(src, dst, ew, batch, msel_dtype=NP_F8):
    """Sort edges by dst, LPT-assign 128-node blocks to cores, sort blocks
    by edge count within each core, and build per-core slot metadata:
    gather indices (srcidx), selection matrices (fp8 M), batchloc."""
    deg = np.bincount(dst, weights=ew.astype(np.float64), minlength=N_NODES)
    deg = deg.astype(np.float32) + np.float32(1.0)
    dinv = (np.float32(1.0) / np.sqrt(deg)).astype(np.float32)
    norm = (dinv[src] * ew * dinv[dst]).astype(np.float32)

    # merge self-loops into the edge list, then sort everything by dst so
    # slots are dst-ordered (narrow selection windows)
    nodes = np.arange(N_NODES, dtype=np.int64)
    alld = np.concatenate([dst, nodes])
    alls = np.concatenate([src, nodes])
    alln = np.concatenate([norm, dinv * dinv])
    order = np.argsort(alld, kind="stable")
    ds = alld[order]
    ss = alls[order]
    ns = alln[order]

    # per-global-block slot ranges (slots sorted by dst, self loops included)
    bnds = np.arange(NGB + 1, dtype=np.int64) * P
    bnds[-1] = N_NODES
    cuts = np.searchsorted(ds, bnds)
    cnt = np.diff(cuts)                         # slots per block

    # LPT assignment of blocks to cores (capacity NPOS each)
    order_blk = np.argsort(-cnt, kind="stable")
    core_tot = np.zeros(NCORES, dtype=np.int64)
    core_blocks = [[] for _ in range(NCORES)]
    for b in order_blk:
        cands = [c for c in range(NCORES) if len(core_blocks[c]) < NPOS]
        c = min(cands, key=lambda c: core_tot[c])
        core_blocks[c].append(b)
        core_tot[c] += cnt[b]
    # within each core: sort by count desc -> positions; pad with -1
    blocks = np.full((NCORES, NPOS), -1, dtype=np.int64)
    for c in range(NCORES):
        bl = sorted(core_blocks[c], key=lambda b: -cnt[b])
        blocks[c, :len(bl)] = bl

    cnt_cp = np.zeros((NCORES, NPOS), dtype=np.int64)
    for c in range(NCORES):
        for p in range(NPOS):
            b = blocks[c, p]
            cnt_cp[c, p] = cnt[b] if b >= 0 else 0
    Cb_list = [max(1, int(-(-cnt_cp[:, p].max() // P))) for p in range(NPOS)]
    coffs = np.concatenate([[0], np.cumsum(Cb_list)]).astype(int)
    TOTC = int(coffs[-1])

    # cross-core dst windows per (position, chunk-pair) -> shared bases
    npairs = [(cb + 1) // 2 for cb in Cb_list]
    minb = [np.full(n, 999, dtype=np.int64) for n in npairs]
    maxb = [np.full(n, -1, dtype=np.int64) for n in npairs]
    dloc = {}
    for c in range(NCORES):
        for p in range(NPOS):
            b = blocks[c, p]
            if b < 0:
                continue
            g0 = int(bnds[b])
            i0, i1 = cuts[b], cuts[b + 1]
            d_all = ds[i0:i1] - g0
            dloc[(c, p)] = d_all
            for q in range(npairs[p]):
                lo, hi = q * 2 * P, min((q + 1) * 2 * P, len(d_all))
                if lo >= len(d_all):
                    continue
                ch = d_all[lo:hi]
                minb[p][q] = min(minb[p][q], ch.min())
                maxb[p][q] = max(maxb[p][q], ch.max())
    span = max((int((mx - mn).max()) + 1
                for mn, mx in zip(minb, maxb) if len(mx) and mx.max() >= 0),
               default=1)
    W = next(w for w in (32, 48, 64, 96, P) if w >= span)
    bases = [np.minimum(np.where(minb[p] > P, 0, minb[p]), P - W).astype(int)
             for p in range(NPOS)]

    srcidx = np.zeros((NCORES, TOTC * P), dtype=np.int64)
    msel = np.zeros((NCORES, P, TOTC, W), dtype=msel_dtype)
    rvec = np.zeros((NCORES, TOTC * P), dtype=np.float32)
    batchloc = np.full((NCORES, P, NPOS), -5.0, dtype=np.float32)

    for c in range(NCORES):
        m2 = np.zeros((TOTC * P, W), dtype=np.float32)
        for p in range(NPOS):
            b = blocks[c, p]
            if b < 0:
                continue
            g0, g1 = int(bnds[b]), int(min(bnds[b + 1], N_NODES))
            i0, i1 = cuts[b], cuts[b + 1]
            s_all = ss[i0:i1]
            d_all = dloc[(c, p)]
            n_all = ns[i0:i1]
            s0 = coffs[p] * P
            k = len(s_all)
            srcidx[c, s0:s0 + k] = s_all
            # quantize norm to the msel dtype; fold the quantization residual
            # into the (private) pre-gathered stream rows via rvec
            nq32 = (n_all * MSCALE).astype(msel_dtype).astype(np.float32)
            with np.errstate(divide="ignore", invalid="ignore"):
                r = np.where(nq32 > 0, n_all * MSCALE / nq32, 0.0)
            d_rel = d_all - np.repeat(bases[p], 2 * P)[:k]
            assert d_rel.min() >= 0 and d_rel.max() < W
            m2[np.arange(s0, s0 + k), d_rel] = nq32
            rvec[c, s0:s0 + k] = r
            batchloc[c, :g1 - g0, p] = batch[g0:g1]
        msel[c] = m2.reshape(TOTC, P, W).transpose(1, 0, 2).astype(msel_dtype)

    return dict(Cb_list=Cb_list, srcidx=srcidx, msel=msel, rvec=rvec,
                batchloc=batchloc.astype(NP_BF16), blocks=blocks,
                W=W, bases=bases)


def _make_sm(table, srcidx_c, rvec_c, msel_c, TOTC, F):
    """Pre-gather table rows, scale by the norm-quantization residual, and
    interleave with the compact selection matrix: sm [128, TOTC, F+W] fp8."""
    W = msel_c.shape[2]
    g = table[srcidx_c].astype(np.float32) * rvec_c[:, None]
    sm = np.empty((P, TOTC, F + W), dtype=NP_F8)
    sm[:, :, :F] = g.astype(NP_F8).reshape(TOTC, P, F).transpose(1, 0, 2)
    sm[:, :, F:] = msel_c
    return sm


def _w_arrange(W):
    F_in, F_out = W.shape
    KT = F_in // P
    return np.ascontiguousarray(
        (W / MSCALE).reshape(KT, P, F_out).transpose(1, 0, 2)).astype(NP_BF16)


def _bias_row(b, F_out):
    r = np.zeros((P, F_out), dtype=NP_BF16)
    r[0, :] = b.astype(NP_BF16)
    return r


def _onehot():
    o = np.zeros((P, P), dtype=NP_BF16)
    o[0, :] = 1
    return o


def _iota64():
    return np.ascontiguousarray(
        np.broadcast_to(np.arange(N_GRAPHS, dtype=NP_BF16), (P, 1, N_GRAPHS)))


def _assemble_h1(houts, blocks, F):
    """hout [128, NPOS, F] per core -> full h1 table [NGB*128, F] fp8."""
    h1 = np.zeros((NGB * P, F), dtype=NP_F8)
    h1v = h1.reshape(NGB, P, F)
    for c in range(NCORES):
        bl = blocks[c]
        val = bl >= 0
        # hout[:, p, :] -> rows of block bl[p]
        h1v[bl[val]] = houts[c].transpose(1, 0, 2)[val]
    return h1


def _run_gcn(x, edge_index, edge_weight, batch, W1, b1, W2, b2, Wl, bl,
             exec_fn=None):
    """Returns out [64,1] fp32.  exec_fn(nc, in_maps) -> list of per-core
    result dicts; defaults to run_bass_kernel_spmd."""
    src = np.asarray(edge_index[0]).astype(np.int64)
    dst = np.asarray(edge_index[1]).astype(np.int64)
    ew = np.asarray(edge_weight).astype(np.float32)
    batch = np.asarray(batch).astype(np.int64)
    x = np.ascontiguousarray(np.asarray(x, dtype=np.float32))

    if exec_fn is None:
        def exec_fn(nc, in_maps):
            r = run_bass_kernel_spmd(nc, in_maps, core_ids=list(range(NCORES)))
            return r.results

    pre = _preprocess(src, dst, ew, batch)
    Cb_list = pre["Cb_list"]
    TOTC = sum(Cb_list)

    nc1 = _build_layer(IN_CH, HID, Cb_list, pre["W"], pre["bases"], layer2=False)
    nc2 = _build_layer(HID, HID, Cb_list, pre["W"], pre["bases"], layer2=True)

    w1 = _w_arrange(np.asarray(W1, dtype=np.float32))
    w2 = _w_arrange(np.asarray(W2, dtype=np.float32))
    br1 = _bias_row(np.asarray(b1, dtype=np.float32), HID)
    br2 = _bias_row(np.asarray(b2, dtype=np.float32), HID)
    oh = _onehot()

    in_maps1 = [dict(sm=_make_sm(x, pre["srcidx"][c], pre["rvec"][c],
                                 pre["msel"][c], TOTC, IN_CH),
                     w=w1, biasrow=br1, onehot=oh)
                for c in range(NCORES)]
    r1 = exec_fn(nc1, in_maps1)
    h1 = _assemble_h1([r1[c]["hout"] for c in range(NCORES)], pre["blocks"], HID)

    io64 = _iota64()
    in_maps2 = [dict(sm=_make_sm(h1, pre["srcidx"][c], pre["rvec"][c],
                                 pre["msel"][c], TOTC, HID),
                     w=w2, biasrow=br2, onehot=oh,
                     batchloc=pre["batchloc"][c], iota64=io64)
                for c in range(NCORES)]
    r2 = exec_fn(nc2, in_maps2)
    pool = np.sum([r2[c]["pout"].astype(np.float64) for c in range(NCORES)],
                  axis=0).astype(np.float32)

    cnt = np.bincount(batch, minlength=N_GRAPHS).astype(np.float32)
    g = pool / np.maximum(cnt, 1.0)[:, None]
    out = (g @ np.asarray(Wl, dtype=np.float32)
           + np.asarray(bl, dtype=np.float32))
    return out.astype(np.float32)


def kernel(**inputs):
    return _run_gcn(
        inputs["x"], inputs["edge_index"], inputs["edge_weight"],
        inputs["batch"], inputs["W1"], inputs["b1"], inputs["W2"],
        inputs["b2"], inputs["Wl"], inputs["bl"])


def _exec_layer(nc, in_maps, bench_iters=0):
    """Execute a compiled layer on the 8 cores via PJRT (same lowering as
    run_bass_kernel_spmd under axon), optionally re-running it
    `bench_iters` times with device-resident inputs to wall-clock the
    execution.  Returns (per-core results list, best_exec_seconds|None)."""
    import time
    import jax
    from jax.experimental.shard_map import shard_map
    from jax.sharding import Mesh, PartitionSpec, NamedSharding
    from concourse import bass2jax, mybir as mb

    bass2jax.install_neuronx_cc_hook()
    n_cores = len(in_maps)
    partition_name = (nc.partition_id_tensor.name if nc.partition_id_tensor
                      else None)
    in_names, out_names, out_avals, zero_outs = [], [], [], []
    for alloc in nc.m.functions[0].allocations:
        if not isinstance(alloc, mb.MemoryLocationSet):
            continue
        name = alloc.memorylocations[0].name
        if alloc.kind == "ExternalInput":
            if name != partition_name:
                in_names.append(name)
        elif alloc.kind == "ExternalOutput":
            out_names.append(name)
            shape = tuple(alloc.tensor_shape)
            dtype = mb.dt.np(alloc.dtype)
            out_avals.append(jax.core.ShapedArray(shape, dtype))
            zero_outs.append(np.zeros(shape, dtype))
    n_params = len(in_names)
    n_outs = len(out_avals)
    all_in_names = list(in_names) + out_names
    if partition_name is not None:
        all_in_names.append(partition_name)

    def _body(*args):
        operands = list(args)
        if partition_name is not None:
            operands.append(bass2jax.partition_id_tensor())
        outs = bass2jax._bass_exec_p.bind(
            *operands,
            out_avals=tuple(out_avals),
            in_names=tuple(all_in_names),
            out_names=tuple(out_names),
            lowering_input_output_aliases=(),
            sim_require_finite=True,
            sim_require_nnan=True,
            nc=nc,
        )
        return tuple(outs)

    devices = jax.devices()[:n_cores]
    mesh = Mesh(np.asarray(devices), ("core",))
    spec = PartitionSpec("core")
    in_specs = (spec,) * (n_params + n_outs)
    out_specs = (spec,) * n_outs
    donate = tuple(range(n_params, n_params + n_outs))
    sharded = jax.jit(
        shard_map(_body, mesh=mesh, in_specs=in_specs, out_specs=out_specs,
                  check_rep=False),
        donate_argnums=donate, keep_unused=True)

    sh = NamedSharding(mesh, spec)
    concat_in = [
        jax.device_put(
            np.concatenate([np.asarray(in_maps[c][nm]) for c in range(n_cores)],
                           axis=0), sh)
        for nm in in_names]
    def put_zeros():
        return [jax.device_put(
                    np.zeros((n_cores * z.shape[0], *z.shape[1:]), z.dtype), sh)
                for z in zero_outs]

    out_arrs = sharded(*concat_in, *put_zeros())
    jax.block_until_ready(out_arrs)
    results = [
        {nm: np.asarray(out_arrs[i]).reshape(n_cores, *out_avals[i].shape)[c]
         for i, nm in enumerate(out_names)}
        for c in range(n_cores)]

    best = None
    for _ in range(bench_iters):
        zs = put_zeros()
        jax.block_until_ready(zs)
        t0 = time.perf_counter()
        o = sharded(*concat_in, *zs)
        jax.block_until_ready(o)
        dt = time.perf_counter() - t0
        best = dt if best is None or dt < best else best
    return results, best



# revision 3
# speedup vs baseline: 1.1368x; 1.1368x over previous
"""Bass/Trainium2 kernel for a 2-layer GCN (GCNConv -> ReLU -> GCNConv ->
ReLU -> global_mean_pool -> Linear), distributed over 8 NeuronCores.

Strategy (graph/data parallel): nodes are split into 391 blocks of 128;
blocks are LPT-assigned to the 8 cores to balance edge counts, and sorted
by edge count within each core so the per-position cross-core max padding
is minimal.  The host pre-gathers each core's per-edge source rows into a
dense fp8 stream in SBUF layout (sequential DMA on device - no SWDGE
gather), interleaved with precomputed compact selection matrices M (fp8,
W=32 dst columns per chunk-pair at a static cross-core-shared base; the
GCN norm x16 is folded into M, weights are scaled 1/16 to compensate, and
the fp8 quantization residual of the norm is folded into the private
stream rows so M quantization cancels exactly).
On device, each 128-dst-node position does (all edges sorted by dst):
  zT[k, base:base+W] += stream[:, j, 128k:...].T @ M[:, j, :]  (fp8 DoubleRow
      into a PSUM bank pre-zeroed by one all-zeros DoubleRow matmul)
  h = sum_k zT[k].T @ W[k] + onehot.T @ biasrow                (bf16, PSUM)
  h = relu(h) -> fp8                                           (Act engine)
Layer 1 writes h1 (fp8) in position-groups; the host reassembles h1,
pre-gathers the layer-2 stream, and launch 2 repeats the same pipeline,
additionally accumulating per-graph pooling sums via one extra matmul per
position.  The final (tiny) mean + [512,1] linear runs on the host.
"""
import sys
sys.path.insert(0, "/opt/trn_rl_repo")

import numpy as np
import ml_dtypes
from contextlib import ExitStack

from concourse import mybir
import concourse.bacc as bacc
import concourse.tile as tile
from concourse.bass_utils import run_bass_kernel_spmd

P = 128
N_NODES = 50000
N_EDGES = 800000
IN_CH = 256
HID = 512
N_GRAPHS = 64
NCORES = 8
NGB = (N_NODES + P - 1) // P       # 391 global blocks of 128 nodes
NPOS = (NGB + NCORES - 1) // NCORES  # 49 positions per core
GRP = 6                            # positions per DMA group

F32 = mybir.dt.float32
BF16 = mybir.dt.bfloat16
F8 = mybir.dt.float8e4
NP_F8 = ml_dtypes.float8_e4m3
NP_BF16 = ml_dtypes.bfloat16

MSCALE = np.float32(16.0)          # norm scale folded into M; W scaled 1/16


def _build_layer(F_in, F_out, Cb_list, W, bases, layer2):
    """Build + compile the bass module for one GCN layer (SPMD, per-core).

    W: width of the compact selection matrices; bases[p][q] is the static
    dst-column base of pair q at position p (shared across cores)."""
    TOTC = sum(Cb_list)
    KT = F_in // P
    coffs = np.concatenate([[0], np.cumsum(Cb_list)]).astype(int)
    # ramped position-group sizes: small first groups shorten pipeline fill,
    # small last groups shorten the compute drain after the final DMA
    head, tail = [1, 1, 2], [2, 1, 1]
    sizes = list(head)
    while sum(sizes) + GRP <= NPOS - sum(tail):
        sizes.append(GRP)
    mid_rem = NPOS - sum(sizes) - sum(tail)
    if mid_rem > 0:
        sizes.append(mid_rem)
    sizes += tail
    gstart = np.concatenate([[0], np.cumsum(sizes)]).astype(int)
    grp_of = np.searchsorted(gstart, np.arange(NPOS), side="right") - 1
    goff = [coffs[gstart[g]] for g in range(len(sizes))]
    gsz = [coffs[gstart[g + 1]] - goff[g] for g in range(len(sizes))]

    nc = bacc.Bacc("TRN2", target_bir_lowering=False, debug=False)
    # stream rows and selection matrices are interleaved per chunk:
    # sm[:, j, :F_in] = gathered rows, sm[:, j, F_in:] = compact M columns
    sm_d = nc.dram_tensor("sm", [P, TOTC, F_in + W], F8, kind="ExternalInput")
    w_d = nc.dram_tensor("w", [P, KT, F_out], BF16, kind="ExternalInput")
    biasrow_d = nc.dram_tensor("biasrow", [P, F_out], BF16, kind="ExternalInput")
    onehot_d = nc.dram_tensor("onehot", [P, P], BF16, kind="ExternalInput")
    if layer2:
        batchloc_d = nc.dram_tensor("batchloc", [P, NPOS], BF16, kind="ExternalInput")
        iota64_d = nc.dram_tensor("iota64", [P, 1, N_GRAPHS], BF16, kind="ExternalInput")
        pout_d = nc.dram_tensor("pout", [N_GRAPHS, F_out], F32, kind="ExternalOutput")
    else:
        hout_d = nc.dram_tensor("hout", [P, NPOS, F_out], F8, kind="ExternalOutput")

    with tile.TileContext(nc) as tc, ExitStack() as ctx:
        const = ctx.enter_context(tc.tile_pool(name="const", bufs=1))
        sg = ctx.enter_context(tc.tile_pool(name="sg", bufs=3))
        hst = ctx.enter_context(tc.tile_pool(name="hst", bufs=2))
        work = ctx.enter_context(tc.tile_pool(name="work", bufs=3))
        zps = ctx.enter_context(tc.tile_pool(name="zps", bufs=2, space="PSUM"))
        hps = ctx.enter_context(tc.tile_pool(name="hps", bufs=2, space="PSUM"))
        if layer2:
            pps = ctx.enter_context(tc.tile_pool(name="pps", bufs=1, space="PSUM"))

        GMAX = max(gsz)
        sg_t = hst_t = None
        # first stream group goes out before the consts so compute can start
        sg_t = sg.tile([P, GMAX, F_in + W], F8, tag="sg")
        nc.sync.dma_start(sg_t[:, :gsz[0], :], sm_d[:, 0:gsz[0], :])

        w_sb = const.tile([P, KT, F_out], BF16)
        nc.sync.dma_start(w_sb[:], w_d[:])
        biasrow_sb = const.tile([P, F_out], BF16)
        nc.sync.dma_start(biasrow_sb[:], biasrow_d[:])
        onehot_sb = const.tile([P, P], BF16)
        nc.sync.dma_start(onehot_sb[:], onehot_d[:])
        czero = const.tile([P, 2, KT * P], F8)
        nc.vector.memset(czero[:], 0.0)
        if layer2:
            batchloc_sb = const.tile([P, NPOS], BF16)
            nc.sync.dma_start(batchloc_sb[:], batchloc_d[:])
            iota64_sb = const.tile([P, 1, N_GRAPHS], BF16)
            nc.sync.dma_start(iota64_sb[:], iota64_d[:])
            pool_ps = pps.tile([N_GRAPHS, F_out], F32)

        for p in range(NPOS):
            g, Cb = int(grp_of[p]), Cb_list[p]
            if p == gstart[g]:
                if p > 0:
                    sg_t = sg.tile([P, GMAX, F_in + W], F8, tag="sg")
                    nc.sync.dma_start(sg_t[:, :gsz[g], :],
                                      sm_d[:, goff[g]:goff[g] + gsz[g], :])
                if not layer2:
                    hst_t = hst.tile([P, max(sizes), F_out], F8, tag="hst")
            o = coffs[p] - goff[g]

            zT_ps = zps.tile([P, KT, P], F32)
            # zero the whole bank with one DoubleRow matmul of zeros
            nc.tensor.matmul(
                zT_ps[:], czero[:, :, :P], czero[:],
                perf_mode=mybir.MatmulPerfMode.DoubleRow,
                start=True, stop=False, skip_group_check=True)
            nlast = (KT - 1, len(bases[p]) - 1)
            for k in range(KT):
                for q, base in enumerate(bases[p]):
                    j = 2 * q
                    stop = (k, q) == nlast
                    if j + 1 < Cb:
                        nc.tensor.matmul(
                            zT_ps[:, k, base:base + W],
                            sg_t[:, o + j:o + j + 2, k * P:(k + 1) * P],
                            sg_t[:, o + j:o + j + 2, F_in:F_in + W],
                            perf_mode=mybir.MatmulPerfMode.DoubleRow,
                            start=False, stop=stop, skip_group_check=True)
                    else:
                        nc.tensor.matmul(
                            zT_ps[:, k, base:base + W],
                            sg_t[:, o + j, k * P:(k + 1) * P],
                            sg_t[:, o + j, F_in:F_in + W],
                            start=False, stop=stop, skip_group_check=True)

            zT_sb = work.tile([P, KT, P], BF16, tag="zT")
            nc.vector.tensor_copy(zT_sb[:], zT_ps[:])

            h_ps = hps.tile([P, F_out], F32)
            for k in range(KT):
                nc.tensor.matmul(h_ps[:], zT_sb[:, k, :], w_sb[:, k, :],
                                 start=(k == 0), stop=False)
            nc.tensor.matmul(h_ps[:], onehot_sb[:], biasrow_sb[:],
                             start=False, stop=True)

            if layer2:
                h_sb = work.tile([P, F_out], BF16, tag="h")
                nc.scalar.activation(h_sb[:], h_ps[:],
                                     mybir.ActivationFunctionType.Relu)
                G = work.tile([P, 1, N_GRAPHS], BF16, tag="G")
                nc.vector.tensor_tensor(
                    out=G[:],
                    in0=batchloc_sb[:, p:p + 1].to_broadcast([P, 1, N_GRAPHS]),
                    in1=iota64_sb[:],
                    op=mybir.AluOpType.is_equal)
                nc.tensor.matmul(pool_ps[:], G[:, 0, :], h_sb[:],
                                 start=(p == 0), stop=(p == NPOS - 1),
                                 skip_group_check=True)
            else:
                nc.scalar.activation(hst_t[:, p - gstart[g], :], h_ps[:],
                                     mybir.ActivationFunctionType.Relu)
                if p == gstart[g + 1] - 1:
                    gcnt = p - gstart[g] + 1
                    nc.sync.dma_start(
                        hout_d[:, gstart[g]:gstart[g] + gcnt, :],
                        hst_t[:, :gcnt, :])

        if layer2:
            p_sb = work.tile([N_GRAPHS, F_out], F32, tag="p")
            nc.vector.tensor_copy(p_sb[:], pool_ps[:])
            nc.sync.dma_start(pout_d[:, :], p_sb[:])

    nc.compile()
    return nc


def _preprocess(src, dst, ew, batch, msel_dtype=NP_F8):
    """Sort edges by dst, LPT-assign 128-node blocks to cores, sort blocks
    by edge count within each core, and build per-core slot metadata:
    gather indices (srcidx), selection matrices (fp8 M), batchloc."""
    deg = np.bincount(dst, weights=ew.astype(np.float64), minlength=N_NODES)
    deg = deg.astype(np.float32) + np.float32(1.0)
    dinv = (np.float32(1.0) / np.sqrt(deg)).astype(np.float32)
    norm = (dinv[src] * ew * dinv[dst]).astype(np.float32)

    # merge self-loops into the edge list, then sort everything by dst so
    # slots are dst-ordered (narrow selection windows)
    nodes = np.arange(N_NODES, dtype=np.int64)
    alld = np.concatenate([dst, nodes])
    alls = np.concatenate([src, nodes])
    alln = np.concatenate([norm, dinv * dinv])
    order = np.argsort(alld, kind="stable")
    ds = alld[order]
    ss = alls[order]
    ns = alln[order]

    # per-global-block slot ranges (slots sorted by dst, self loops included)
    bnds = np.arange(NGB + 1, dtype=np.int64) * P
    bnds[-1] = N_NODES
    cuts = np.searchsorted(ds, bnds)
    cnt = np.diff(cuts)                         # slots per block

    # LPT assignment of blocks to cores (capacity NPOS each)
    order_blk = np.argsort(-cnt, kind="stable")
    core_tot = np.zeros(NCORES, dtype=np.int64)
    core_blocks = [[] for _ in range(NCORES)]
    for b in order_blk:
        cands = [c for c in range(NCORES) if len(core_blocks[c]) < NPOS]
        c = min(cands, key=lambda c: core_tot[c])
        core_blocks[c].append(b)
        core_tot[c] += cnt[b]
    # within each core: sort by count desc -> positions; pad with -1
    blocks = np.full((NCORES, NPOS), -1, dtype=np.int64)
    for c in range(NCORES):
        bl = sorted(core_blocks[c], key=lambda b: -cnt[b])
        blocks[c, :len(bl)] = bl

    cnt_cp = np.zeros((NCORES, NPOS), dtype=np.int64)
    for c in range(NCORES):
        for p in range(NPOS):
            b = blocks[c, p]
            cnt_cp[c, p] = cnt[b] if b >= 0 else 0
    Cb_list = [max(1, int(-(-cnt_cp[:, p].max() // P))) for p in range(NPOS)]
    coffs = np.concatenate([[0], np.cumsum(Cb_list)]).astype(int)
    TOTC = int(coffs[-1])

    # cross-core dst windows per (position, chunk-pair) -> shared bases
    npairs = [(cb + 1) // 2 for cb in Cb_list]
    minb = [np.full(n, 999, dtype=np.int64) for n in npairs]
    maxb = [np.full(n, -1, dtype=np.int64) for n in npairs]
    dloc = {}
    for c in range(NCORES):
        for p in range(NPOS):
            b = blocks[c, p]
            if b < 0:
                continue
            g0 = int(bnds[b])
            i0, i1 = cuts[b], cuts[b + 1]
            d_all = ds[i0:i1] - g0
            dloc[(c, p)] = d_all
            for q in range(npairs[p]):
                lo, hi = q * 2 * P, min((q + 1) * 2 * P, len(d_all))
                if lo >= len(d_all):
                    continue
                ch = d_all[lo:hi]
                minb[p][q] = min(minb[p][q], ch.min())
                maxb[p][q] = max(maxb[p][q], ch.max())
    span = max((int((mx - mn).max()) + 1
                for mn, mx in zip(minb, maxb) if len(mx) and mx.max() >= 0),
               default=1)
    W = next(w for w in (32, 48, 64, 96, P) if w >= span)
    bases = [np.minimum(np.where(minb[p] > P, 0, minb[p]), P - W).astype(int)
             for p in range(NPOS)]

    srcidx = np.zeros((NCORES, TOTC * P), dtype=np.int64)
    msel = np.zeros((NCORES, P, TOTC, W), dtype=msel_dtype)
    rvec = np.zeros((NCORES, TOTC * P), dtype=np.float32)
    batchloc = np.full((NCORES, P, NPOS), -5.0, dtype=np.float32)

    for c in range(NCORES):
        m2 = np.zeros((TOTC * P, W), dtype=np.float32)
        for p in range(NPOS):
            b = blocks[c, p]
            if b < 0:
                continue
            g0, g1 = int(bnds[b]), int(min(bnds[b + 1], N_NODES))
            i0, i1 = cuts[b], cuts[b + 1]
            s_all = ss[i0:i1]
            d_all = dloc[(c, p)]
            n_all = ns[i0:i1]
            s0 = coffs[p] * P
            k = len(s_all)
            srcidx[c, s0:s0 + k] = s_all
            # quantize norm to the msel dtype; fold the quantization residual
            # into the (private) pre-gathered stream rows via rvec
            nq32 = (n_all * MSCALE).astype(msel_dtype).astype(np.float32)
            with np.errstate(divide="ignore", invalid="ignore"):
                r = np.where(nq32 > 0, n_all * MSCALE / nq32, 0.0)
            d_rel = d_all - np.repeat(bases[p], 2 * P)[:k]
            assert d_rel.min() >= 0 and d_rel.max() < W
            m2[np.arange(s0, s0 + k), d_rel] = nq32
            rvec[c, s0:s0 + k] = r
            batchloc[c, :g1 - g0, p] = batch[g0:g1]
        msel[c] = m2.reshape(TOTC, P, W).transpose(1, 0, 2).astype(msel_dtype)

    return dict(Cb_list=Cb_list, srcidx=srcidx, msel=msel, rvec=rvec,
                batchloc=batchloc.astype(NP_BF16), blocks=blocks,
                W=W, bases=bases)


def _make_sm(table, srcidx_c, rvec_c, msel_c, TOTC, F):
    """Pre-gather table rows, scale by the norm-quantization residual, and
    interleave with the compact selection matrix: sm [128, TOTC, F+W] fp8."""
    W = msel_c.shape[2]
    g = table[srcidx_c].astype(np.float32) * rvec_c[:, None]
    sm = np.empty((P, TOTC, F + W), dtype=NP_F8)
    sm[:, :, :F] = g.astype(NP_F8).reshape(TOTC, P, F).transpose(1, 0, 2)
    sm[:, :, F:] = msel_c
    return sm


def _w_arrange(W):
    F_in, F_out = W.shape
    KT = F_in // P
    return np.ascontiguousarray(
        (W / MSCALE).reshape(KT, P, F_out).transpose(1, 0, 2)).astype(NP_BF16)


def _bias_row(b, F_out):
    r = np.zeros((P, F_out), dtype=NP_BF16)
    r[0, :] = b.astype(NP_BF16)
    return r


def _onehot():
    o = np.zeros((P, P), dtype=NP_BF16)
    o[0, :] = 1
    return o


def _iota64():
    return np.ascontiguousarray(
        np.broadcast_to(np.arange(N_GRAPHS, dtype=NP_BF16), (P, 1, N_GRAPHS)))


def _assemble_h1(houts, blocks, F):
    """hout [128, NPOS, F] per core -> full h1 table [NGB*128, F] fp8."""
    h1 = np.zeros((NGB * P, F), dtype=NP_F8)
    h1v = h1.reshape(NGB, P, F)
    for c in range(NCORES):
        bl = blocks[c]
        val = bl >= 0
        # hout[:, p, :] -> rows of block bl[p]
        h1v[bl[val]] = houts[c].transpose(1, 0, 2)[val]
    return h1


def _run_gcn(x, edge_index, edge_weight, batch, W1, b1, W2, b2, Wl, bl,
             exec_fn=None):
    """Returns out [64,1] fp32.  exec_fn(nc, in_maps) -> list of per-core
    result dicts; defaults to run_bass_kernel_spmd."""
    src = np.asarray(edge_index[0]).astype(np.int64)
    dst = np.asarray(edge_index[1]).astype(np.int64)
    ew = np.asarray(edge_weight).astype(np.float32)
    batch = np.asarray(batch).astype(np.int64)
    x = np.ascontiguousarray(np.asarray(x, dtype=np.float32))

    if exec_fn is None:
        def exec_fn(nc, in_maps):
            r = run_bass_kernel_spmd(nc, in_maps, core_ids=list(range(NCORES)))
            return r.results

    pre = _preprocess(src, dst, ew, batch)
    Cb_list = pre["Cb_list"]
    TOTC = sum(Cb_list)

    nc1 = _build_layer(IN_CH, HID, Cb_list, pre["W"], pre["bases"], layer2=False)
    nc2 = _build_layer(HID, HID, Cb_list, pre["W"], pre["bases"], layer2=True)

    w1 = _w_arrange(np.asarray(W1, dtype=np.float32))
    w2 = _w_arrange(np.asarray(W2, dtype=np.float32))
    br1 = _bias_row(np.asarray(b1, dtype=np.float32), HID)
    br2 = _bias_row(np.asarray(b2, dtype=np.float32), HID)
    oh = _onehot()

    in_maps1 = [dict(sm=_make_sm(x, pre["srcidx"][c], pre["rvec"][c],
                                 pre["msel"][c], TOTC, IN_CH),
                     w=w1, biasrow=br1, onehot=oh)
                for c in range(NCORES)]
    r1 = exec_fn(nc1, in_maps1)
    h1 = _assemble_h1([r1[c]["hout"] for c in range(NCORES)], pre["blocks"], HID)

    io64 = _iota64()
    in_maps2 = [dict(sm=_make_sm(h1, pre["srcidx"][c], pre["rvec"][c],
                                 pre["msel"][c], TOTC, HID),
                     w=w2, biasrow=br2, onehot=oh,
                     batchloc=pre["batchloc"][c], iota64=io64)
                for c in range(NCORES)]
    r2 = exec_fn(nc2, in_maps2)
    pool = np.sum([r2[c]["pout"].astype(np.float64) for c in range(NCORES)],
                  axis=0).astype(np.float32)

    cnt = np.bincount(batch, minlength=N_GRAPHS).astype(np.float32)
    g = pool / np.maximum(cnt, 1.0)[:, None]
    out = (g @ np.asarray(Wl, dtype=np.float32)
           + np.asarray(bl, dtype=np.float32))
    return out.astype(np.float32)


def kernel(**inputs):
    return _run_gcn(
        inputs["x"], inputs["edge_index"], inputs["edge_weight"],
        inputs["batch"], inputs["W1"], inputs["b1"], inputs["W2"],
        inputs["b2"], inputs["Wl"], inputs["bl"])


def _exec_layer(nc, in_maps, bench_iters=0):
    """Execute a compiled layer on the 8 cores via PJRT (same lowering as
    run_bass_kernel_spmd under axon), optionally re-running it
    `bench_iters` times with device-resident inputs to wall-clock the
    execution.  Returns (per-core results list, best_exec_seconds|None)."""
    import time
    import jax
    from jax.experimental.shard_map import shard_map
    from jax.sharding import Mesh, PartitionSpec, NamedSharding
    from concourse import bass2jax, mybir as mb

    bass2jax.install_neuronx_cc_hook()
    n_cores = len(in_maps)
    partition_name = (nc.partition_id_tensor.name if nc.partition_id_tensor
                      else None)
    in_names, out_names, out_avals, zero_outs = [], [], [], []
    for alloc in nc.m.functions[0].allocations:
        if not isinstance(alloc, mb.MemoryLocationSet):
            continue
        name = alloc.memorylocations[0].name
        if alloc.kind == "ExternalInput":
            if name != partition_name:
                in_names.append(name)
        elif alloc.kind == "ExternalOutput":
            out_names.append(name)
            shape = tuple(alloc.tensor_shape)
            dtype = mb.dt.np(alloc.dtype)
            out_avals.append(jax.core.ShapedArray(shape, dtype))
            zero_outs.append(np.zeros(shape, dtype))
    n_params = len(in_names)
    n_outs = len(out_avals)
    all_in_names = list(in_names) + out_names
    if partition_name is not None:
        all_in_names.append(partition_name)

    def _body(*args):
        operands = list(args)
        if partition_name is not None:
            operands.append(bass2jax.partition_id_tensor())
        outs = bass2jax._bass_exec_p.bind(
            *operands,
            out_avals=tuple(out_avals),
            in_names=tuple(all_in_names),
            out_names=tuple(out_names),
            lowering_input_output_aliases=(),
            sim_require_finite=True,
            sim_require_nnan=True,
            nc=nc,
        )
        return tuple(outs)

    devices = jax.devices()[:n_cores]
    mesh = Mesh(np.asarray(devices), ("core",))
    spec = PartitionSpec("core")
    in_specs = (spec,) * (n_params + n_outs)
    out_specs = (spec,) * n_outs
    donate = tuple(range(n_params, n_params + n_outs))
    sharded = jax.jit(
        shard_map(_body, mesh=mesh, in_specs=in_specs, out_specs=out_specs,
                  check_rep=False),
        donate_argnums=donate, keep_unused=True)

    sh = NamedSharding(mesh, spec)
    concat_in = [
        jax.device_put(
            np.concatenate([np.asarray(in_maps[c][nm]) for c in range(n_cores)],
                           axis=0), sh)
        for nm in in_names]
    def put_zeros():
        return [jax.device_put(
                    np.zeros((n_cores * z.shape[0], *z.shape[1:]), z.dtype), sh)
                for z in zero_outs]

    out_arrs = sharded(*concat_in, *put_zeros())
    jax.block_until_ready(out_arrs)
    results = [
        {nm: np.asarray(out_arrs[i]).reshape(n_cores, *out_avals[i].shape)[c]
         for i, nm in enumerate(out_names)}
        for c in range(n_cores)]

    best = None
    for _ in range(bench_iters):
        zs = put_zeros()
        jax.block_until_ready(zs)
        t0 = time.perf_counter()
        o = sharded(*concat_in, *zs)
        jax.block_until_ready(o)
        dt = time.perf_counter() - t0
        best = dt if best is None or dt < best else best
    return results, best



# revision 5
# speedup vs baseline: 1.2780x; 1.1242x over previous
"""Bass/Trainium2 kernel for a 2-layer GCN (GCNConv -> ReLU -> GCNConv ->
ReLU -> global_mean_pool -> Linear), distributed over 8 NeuronCores.

Strategy (graph/data parallel): nodes are split into 391 blocks of 128;
blocks are LPT-assigned to the 8 cores to balance edge counts, and sorted
by edge count within each core so the per-position cross-core max padding
is minimal.  The host pre-gathers each core's per-edge source rows into a
dense fp8 stream in SBUF layout (sequential DMA on device - no SWDGE
gather), interleaved with precomputed compact selection matrices M (fp8,
W=32 dst columns per chunk-pair at a static cross-core-shared base; the
GCN norm x16 is folded into M, weights are scaled 1/16 to compensate, and
the fp8 quantization residual of the norm is folded into the private
stream rows so M quantization cancels exactly).
On device, each 128-dst-node position does (all edges sorted by dst):
  zT[k, base:base+W] += stream[:, j, 128k:...].T @ M[:, j, :]  (fp8 DoubleRow
      into a PSUM bank pre-zeroed by one all-zeros DoubleRow matmul)
  h = sum_k zT[k].T @ W[k] + onehot.T @ biasrow                (bf16, PSUM)
  h = relu(h) -> fp8                                           (Act engine)
Layer 1 writes h1 (fp8) in position-groups; the host reassembles h1,
pre-gathers the layer-2 stream, and launch 2 repeats the same pipeline,
additionally accumulating per-graph pooling sums via one extra matmul per
position.  The final (tiny) mean + [512,1] linear runs on the host.
"""
import sys
sys.path.insert(0, "/opt/trn_rl_repo")

import numpy as np
import ml_dtypes
from contextlib import ExitStack

from concourse import mybir
import concourse.bacc as bacc
import concourse.tile as tile
from concourse.bass_utils import run_bass_kernel_spmd

P = 128
N_NODES = 50000
N_EDGES = 800000
IN_CH = 256
HID = 512
N_GRAPHS = 64
NCORES = 8
NGB = (N_NODES + P - 1) // P       # 391 global blocks of 128 nodes
NPOS = (NGB + NCORES - 1) // NCORES  # 49 positions per core
GRP1 = 6                           # positions per DMA group, layer 1
GRP2 = 5                           # positions per DMA group, layer 2

F32 = mybir.dt.float32
BF16 = mybir.dt.bfloat16
F8 = mybir.dt.float8e4
NP_F8 = ml_dtypes.float8_e4m3
NP_BF16 = ml_dtypes.bfloat16

MSCALE = np.float32(16.0)          # norm scale folded into M; W scaled 1/16


def _build_layer(F_in, F_out, Cb_list, W, bases, layer2):
    """Build + compile the bass module for one GCN layer (SPMD, per-core).

    W: width of the compact selection matrices; bases[p][q] is the static
    dst-column base of pair q at position p (shared across cores)."""
    TOTC = sum(Cb_list)
    KT = F_in // P
    coffs = np.concatenate([[0], np.cumsum(Cb_list)]).astype(int)
    # ramped position-group sizes: small first groups shorten pipeline fill,
    # small last groups shorten the compute drain after the final DMA
    GRP = GRP2 if layer2 else GRP1
    head, tail = [1, 1, 2], [2, 1, 1]
    sizes = list(head)
    while sum(sizes) + GRP <= NPOS - sum(tail):
        sizes.append(GRP)
    mid_rem = NPOS - sum(sizes) - sum(tail)
    if mid_rem > 0:
        sizes.append(mid_rem)
    sizes += tail
    gstart = np.concatenate([[0], np.cumsum(sizes)]).astype(int)
    grp_of = np.searchsorted(gstart, np.arange(NPOS), side="right") - 1
    goff = [coffs[gstart[g]] for g in range(len(sizes))]
    gsz = [coffs[gstart[g + 1]] - goff[g] for g in range(len(sizes))]

    nc = bacc.Bacc("TRN2", target_bir_lowering=False, debug=False)
    # stream rows and selection matrices are interleaved per chunk:
    # sm[:, j, :F_in] = gathered rows, sm[:, j, F_in:] = compact M columns
    sm_d = nc.dram_tensor("sm", [P, TOTC, F_in + W], F8, kind="ExternalInput")
    w_d = nc.dram_tensor("w", [P, KT, F_out], BF16, kind="ExternalInput")
    biasrow_d = nc.dram_tensor("biasrow", [P, F_out], BF16, kind="ExternalInput")
    onehot_d = nc.dram_tensor("onehot", [P, P], BF16, kind="ExternalInput")
    if layer2:
        batchloc_d = nc.dram_tensor("batchloc", [P, NPOS], BF16, kind="ExternalInput")
        iota64_d = nc.dram_tensor("iota64", [P, 1, N_GRAPHS], BF16, kind="ExternalInput")
        pout_d = nc.dram_tensor("pout", [N_GRAPHS, F_out], F32, kind="ExternalOutput")
    else:
        hout_d = nc.dram_tensor("hout", [P, NPOS, F_out], F8, kind="ExternalOutput")

    with tile.TileContext(nc) as tc, ExitStack() as ctx:
        const = ctx.enter_context(tc.tile_pool(name="const", bufs=1))
        sg = ctx.enter_context(tc.tile_pool(name="sg", bufs=3))
        hst = ctx.enter_context(tc.tile_pool(name="hst", bufs=2))
        work = ctx.enter_context(tc.tile_pool(name="work", bufs=3))
        zps = ctx.enter_context(tc.tile_pool(name="zps", bufs=2, space="PSUM"))
        hps = ctx.enter_context(tc.tile_pool(name="hps", bufs=2, space="PSUM"))
        if layer2:
            pps = ctx.enter_context(tc.tile_pool(name="pps", bufs=1, space="PSUM"))

        GMAX = max(gsz)
        sg_t = hst_t = None
        # first stream group goes out before the consts so compute can start
        sg_t = sg.tile([P, GMAX, F_in + W], F8, tag="sg")
        nc.sync.dma_start(sg_t[:, :gsz[0], :], sm_d[:, 0:gsz[0], :])

        w_sb = const.tile([P, KT, F_out], BF16)
        nc.sync.dma_start(w_sb[:], w_d[:])
        biasrow_sb = const.tile([P, F_out], BF16)
        nc.sync.dma_start(biasrow_sb[:], biasrow_d[:])
        onehot_sb = const.tile([P, P], BF16)
        nc.sync.dma_start(onehot_sb[:], onehot_d[:])
        czero = const.tile([P, 2, KT * P], F8)
        nc.vector.memset(czero[:], 0.0)
        if layer2:
            batchloc_sb = const.tile([P, NPOS], BF16)
            nc.sync.dma_start(batchloc_sb[:], batchloc_d[:])
            iota64_sb = const.tile([P, 1, N_GRAPHS], BF16)
            nc.sync.dma_start(iota64_sb[:], iota64_d[:])
            pool_ps = pps.tile([N_GRAPHS, F_out], F32)

        for p in range(NPOS):
            g, Cb = int(grp_of[p]), Cb_list[p]
            if p == gstart[g]:
                if p > 0:
                    sg_t = sg.tile([P, GMAX, F_in + W], F8, tag="sg")
                    nc.sync.dma_start(sg_t[:, :gsz[g], :],
                                      sm_d[:, goff[g]:goff[g] + gsz[g], :])
                if not layer2:
                    hst_t = hst.tile([P, max(sizes), F_out], F8, tag="hst")
            o = coffs[p] - goff[g]

            zT_ps = zps.tile([P, KT, P], F32)
            # zero the whole bank with one DoubleRow matmul of zeros
            nc.tensor.matmul(
                zT_ps[:], czero[:, :, :P], czero[:],
                perf_mode=mybir.MatmulPerfMode.DoubleRow,
                start=True, stop=False, skip_group_check=True)
            nlast = (KT - 1, len(bases[p]) - 1)
            for k in range(KT):
                for q, base in enumerate(bases[p]):
                    j = 2 * q
                    stop = (k, q) == nlast
                    if j + 1 < Cb:
                        nc.tensor.matmul(
                            zT_ps[:, k, base:base + W],
                            sg_t[:, o + j:o + j + 2, k * P:(k + 1) * P],
                            sg_t[:, o + j:o + j + 2, F_in:F_in + W],
                            perf_mode=mybir.MatmulPerfMode.DoubleRow,
                            start=False, stop=stop, skip_group_check=True)
                    else:
                        nc.tensor.matmul(
                            zT_ps[:, k, base:base + W],
                            sg_t[:, o + j, k * P:(k + 1) * P],
                            sg_t[:, o + j, F_in:F_in + W],
                            start=False, stop=stop, skip_group_check=True)

            zT_sb = work.tile([P, KT, P], BF16, tag="zT")
            nc.vector.tensor_copy(zT_sb[:], zT_ps[:])

            h_ps = hps.tile([P, F_out], F32)
            for k in range(KT):
                nc.tensor.matmul(h_ps[:], zT_sb[:, k, :], w_sb[:, k, :],
                                 start=(k == 0), stop=False)
            nc.tensor.matmul(h_ps[:], onehot_sb[:], biasrow_sb[:],
                             start=False, stop=True)

            if layer2:
                h_sb = work.tile([P, F_out], BF16, tag="h")
                nc.scalar.activation(h_sb[:], h_ps[:],
                                     mybir.ActivationFunctionType.Relu)
                G = work.tile([P, 1, N_GRAPHS], BF16, tag="G")
                nc.vector.tensor_tensor(
                    out=G[:],
                    in0=batchloc_sb[:, p:p + 1].to_broadcast([P, 1, N_GRAPHS]),
                    in1=iota64_sb[:],
                    op=mybir.AluOpType.is_equal)
                nc.tensor.matmul(pool_ps[:], G[:, 0, :], h_sb[:],
                                 start=(p == 0), stop=(p == NPOS - 1),
                                 skip_group_check=True)
            else:
                nc.scalar.activation(hst_t[:, p - gstart[g], :], h_ps[:],
                                     mybir.ActivationFunctionType.Relu)
                if p == gstart[g + 1] - 1:
                    gcnt = p - gstart[g] + 1
                    nc.sync.dma_start(
                        hout_d[:, gstart[g]:gstart[g] + gcnt, :],
                        hst_t[:, :gcnt, :])

        if layer2:
            p_sb = work.tile([N_GRAPHS, F_out], F32, tag="p")
            nc.vector.tensor_copy(p_sb[:], pool_ps[:])
            nc.sync.dma_start(pout_d[:, :], p_sb[:])

    nc.compile()
    return nc


def _preprocess(src, dst, ew, batch, msel_dtype=NP_F8):
    """Sort edges by dst, LPT-assign 128-node blocks to cores, sort blocks
    by edge count within each core, and build per-core slot metadata:
    gather indices (srcidx), selection matrices (fp8 M), batchloc."""
    deg = np.bincount(dst, weights=ew.astype(np.float64), minlength=N_NODES)
    deg = deg.astype(np.float32) + np.float32(1.0)
    dinv = (np.float32(1.0) / np.sqrt(deg)).astype(np.float32)
    norm = (dinv[src] * ew * dinv[dst]).astype(np.float32)

    # merge self-loops into the edge list, then sort everything by dst so
    # slots are dst-ordered (narrow selection windows)
    nodes = np.arange(N_NODES, dtype=np.int64)
    alld = np.concatenate([dst, nodes])
    alls = np.concatenate([src, nodes])
    alln = np.concatenate([norm, dinv * dinv])
    order = np.argsort(alld, kind="stable")
    ds = alld[order]
    ss = alls[order]
    ns = alln[order]

    # per-global-block slot ranges (slots sorted by dst, self loops included)
    bnds = np.arange(NGB + 1, dtype=np.int64) * P
    bnds[-1] = N_NODES
    cuts = np.searchsorted(ds, bnds)
    cnt = np.diff(cuts)                         # slots per block

    # LPT assignment of blocks to cores (capacity NPOS each)
    order_blk = np.argsort(-cnt, kind="stable")
    core_tot = np.zeros(NCORES, dtype=np.int64)
    core_blocks = [[] for _ in range(NCORES)]
    for b in order_blk:
        cands = [c for c in range(NCORES) if len(core_blocks[c]) < NPOS]
        c = min(cands, key=lambda c: core_tot[c])
        core_blocks[c].append(b)
        core_tot[c] += cnt[b]
    # within each core: sort by count desc -> positions; pad with -1
    blocks = np.full((NCORES, NPOS), -1, dtype=np.int64)
    for c in range(NCORES):
        bl = sorted(core_blocks[c], key=lambda b: -cnt[b])
        blocks[c, :len(bl)] = bl

    cnt_cp = np.zeros((NCORES, NPOS), dtype=np.int64)
    for c in range(NCORES):
        for p in range(NPOS):
            b = blocks[c, p]
            cnt_cp[c, p] = cnt[b] if b >= 0 else 0
    Cb_list = [max(1, int(-(-cnt_cp[:, p].max() // P))) for p in range(NPOS)]
    coffs = np.concatenate([[0], np.cumsum(Cb_list)]).astype(int)
    TOTC = int(coffs[-1])

    # cross-core dst windows per (position, chunk-pair) -> shared bases
    npairs = [(cb + 1) // 2 for cb in Cb_list]
    minb = [np.full(n, 999, dtype=np.int64) for n in npairs]
    maxb = [np.full(n, -1, dtype=np.int64) for n in npairs]
    dloc = {}
    for c in range(NCORES):
        for p in range(NPOS):
            b = blocks[c, p]
            if b < 0:
                continue
            g0 = int(bnds[b])
            i0, i1 = cuts[b], cuts[b + 1]
            d_all = ds[i0:i1] - g0
            dloc[(c, p)] = d_all
            for q in range(npairs[p]):
                lo, hi = q * 2 * P, min((q + 1) * 2 * P, len(d_all))
                if lo >= len(d_all):
                    continue
                ch = d_all[lo:hi]
                minb[p][q] = min(minb[p][q], ch.min())
                maxb[p][q] = max(maxb[p][q], ch.max())
    span = max((int((mx - mn).max()) + 1
                for mn, mx in zip(minb, maxb) if len(mx) and mx.max() >= 0),
               default=1)
    W = next(w for w in (32, 48, 64, 96, P) if w >= span)
    bases = [np.minimum(np.where(minb[p] > P, 0, minb[p]), P - W).astype(int)
             for p in range(NPOS)]

    srcidx = np.zeros((NCORES, TOTC * P), dtype=np.int64)
    msel = np.zeros((NCORES, P, TOTC, W), dtype=msel_dtype)
    rvec = np.zeros((NCORES, TOTC * P), dtype=np.float32)
    batchloc = np.full((NCORES, P, NPOS), -5.0, dtype=np.float32)

    for c in range(NCORES):
        m2 = np.zeros((TOTC * P, W), dtype=np.float32)
        for p in range(NPOS):
            b = blocks[c, p]
            if b < 0:
                continue
            g0, g1 = int(bnds[b]), int(min(bnds[b + 1], N_NODES))
            i0, i1 = cuts[b], cuts[b + 1]
            s_all = ss[i0:i1]
            d_all = dloc[(c, p)]
            n_all = ns[i0:i1]
            s0 = coffs[p] * P
            k = len(s_all)
            srcidx[c, s0:s0 + k] = s_all
            # quantize norm to the msel dtype; fold the quantization residual
            # into the (private) pre-gathered stream rows via rvec
            nq32 = (n_all * MSCALE).astype(msel_dtype).astype(np.float32)
            with np.errstate(divide="ignore", invalid="ignore"):
                r = np.where(nq32 > 0, n_all * MSCALE / nq32, 0.0)
            d_rel = d_all - np.repeat(bases[p], 2 * P)[:k]
            assert d_rel.min() >= 0 and d_rel.max() < W
            m2[np.arange(s0, s0 + k), d_rel] = nq32
            rvec[c, s0:s0 + k] = r
            batchloc[c, :g1 - g0, p] = batch[g0:g1]
        msel[c] = m2.reshape(TOTC, P, W).transpose(1, 0, 2).astype(msel_dtype)

    return dict(Cb_list=Cb_list, srcidx=srcidx, msel=msel, rvec=rvec,
                batchloc=batchloc.astype(NP_BF16), blocks=blocks,
                W=W, bases=bases)


def _make_sm(table, srcidx_c, rvec_c, msel_c, TOTC, F):
    """Pre-gather table rows, scale by the norm-quantization residual, and
    interleave with the compact selection matrix: sm [128, TOTC, F+W] fp8."""
    W = msel_c.shape[2]
    g = table[srcidx_c].astype(np.float32) * rvec_c[:, None]
    sm = np.empty((P, TOTC, F + W), dtype=NP_F8)
    sm[:, :, :F] = g.astype(NP_F8).reshape(TOTC, P, F).transpose(1, 0, 2)
    sm[:, :, F:] = msel_c
    return sm


def _w_arrange(W):
    F_in, F_out = W.shape
    KT = F_in // P
    return np.ascontiguousarray(
        (W / MSCALE).reshape(KT, P, F_out).transpose(1, 0, 2)).astype(NP_BF16)


def _bias_row(b, F_out):
    r = np.zeros((P, F_out), dtype=NP_BF16)
    r[0, :] = b.astype(NP_BF16)
    return r


def _onehot():
    o = np.zeros((P, P), dtype=NP_BF16)
    o[0, :] = 1
    return o


def _iota64():
    return np.ascontiguousarray(
        np.broadcast_to(np.arange(N_GRAPHS, dtype=NP_BF16), (P, 1, N_GRAPHS)))


def _assemble_h1(houts, blocks, F):
    """hout [128, NPOS, F] per core -> full h1 table [NGB*128, F] fp8."""
    h1 = np.zeros((NGB * P, F), dtype=NP_F8)
    h1v = h1.reshape(NGB, P, F)
    for c in range(NCORES):
        bl = blocks[c]
        val = bl >= 0
        # hout[:, p, :] -> rows of block bl[p]
        h1v[bl[val]] = houts[c].transpose(1, 0, 2)[val]
    return h1


def _run_gcn(x, edge_index, edge_weight, batch, W1, b1, W2, b2, Wl, bl,
             exec_fn=None):
    """Returns out [64,1] fp32.  exec_fn(nc, in_maps) -> list of per-core
    result dicts; defaults to run_bass_kernel_spmd."""
    src = np.asarray(edge_index[0]).astype(np.int64)
    dst = np.asarray(edge_index[1]).astype(np.int64)
    ew = np.asarray(edge_weight).astype(np.float32)
    batch = np.asarray(batch).astype(np.int64)
    x = np.ascontiguousarray(np.asarray(x, dtype=np.float32))

    if exec_fn is None:
        def exec_fn(nc, in_maps):
            r = run_bass_kernel_spmd(nc, in_maps, core_ids=list(range(NCORES)))
            return r.results

    pre = _preprocess(src, dst, ew, batch)
    Cb_list = pre["Cb_list"]
    TOTC = sum(Cb_list)

    nc1 = _build_layer(IN_CH, HID, Cb_list, pre["W"], pre["bases"], layer2=False)
    nc2 = _build_layer(HID, HID, Cb_list, pre["W"], pre["bases"], layer2=True)

    w1 = _w_arrange(np.asarray(W1, dtype=np.float32))
    w2 = _w_arrange(np.asarray(W2, dtype=np.float32))
    br1 = _bias_row(np.asarray(b1, dtype=np.float32), HID)
    br2 = _bias_row(np.asarray(b2, dtype=np.float32), HID)
    oh = _onehot()

    in_maps1 = [dict(sm=_make_sm(x, pre["srcidx"][c], pre["rvec"][c],
                                 pre["msel"][c], TOTC, IN_CH),
                     w=w1, biasrow=br1, onehot=oh)
                for c in range(NCORES)]
    r1 = exec_fn(nc1, in_maps1)
    h1 = _assemble_h1([r1[c]["hout"] for c in range(NCORES)], pre["blocks"], HID)

    io64 = _iota64()
    in_maps2 = [dict(sm=_make_sm(h1, pre["srcidx"][c], pre["rvec"][c],
                                 pre["msel"][c], TOTC, HID),
                     w=w2, biasrow=br2, onehot=oh,
                     batchloc=pre["batchloc"][c], iota64=io64)
                for c in range(NCORES)]
    r2 = exec_fn(nc2, in_maps2)
    pool = np.sum([r2[c]["pout"].astype(np.float64) for c in range(NCORES)],
                  axis=0).astype(np.float32)

    cnt = np.bincount(batch, minlength=N_GRAPHS).astype(np.float32)
    g = pool / np.maximum(cnt, 1.0)[:, None]
    out = (g @ np.asarray(Wl, dtype=np.float32)
           + np.asarray(bl, dtype=np.float32))
    return out.astype(np.float32)


def kernel(**inputs):
    return _run_gcn(
        inputs["x"], inputs["edge_index"], inputs["edge_weight"],
        inputs["batch"], inputs["W1"], inputs["b1"], inputs["W2"],
        inputs["b2"], inputs["Wl"], inputs["bl"])


def _exec_layer(nc, in_maps, bench_iters=0):
    """Execute a compiled layer on the 8 cores via PJRT (same lowering as
    run_bass_kernel_spmd under axon), optionally re-running it
    `bench_iters` times with device-resident inputs to wall-clock the
    execution.  Returns (per-core results list, best_exec_seconds|None)."""
    import time
    import jax
    from jax.experimental.shard_map import shard_map
    from jax.sharding import Mesh, PartitionSpec, NamedSharding
    from concourse import bass2jax, mybir as mb

    bass2jax.install_neuronx_cc_hook()
    n_cores = len(in_maps)
    partition_name = (nc.partition_id_tensor.name if nc.partition_id_tensor
                      else None)
    in_names, out_names, out_avals, zero_outs = [], [], [], []
    for alloc in nc.m.functions[0].allocations:
        if not isinstance(alloc, mb.MemoryLocationSet):
            continue
        name = alloc.memorylocations[0].name
        if alloc.kind == "ExternalInput":
            if name != partition_name:
                in_names.append(name)
        elif alloc.kind == "ExternalOutput":
            out_names.append(name)
            shape = tuple(alloc.tensor_shape)
            dtype = mb.dt.np(alloc.dtype)
            out_avals.append(jax.core.ShapedArray(shape, dtype))
            zero_outs.append(np.zeros(shape, dtype))
    n_params = len(in_names)
    n_outs = len(out_avals)
    all_in_names = list(in_names) + out_names
    if partition_name is not None:
        all_in_names.append(partition_name)

    def _body(*args):
        operands = list(args)
        if partition_name is not None:
            operands.append(bass2jax.partition_id_tensor())
        outs = bass2jax._bass_exec_p.bind(
            *operands,
            out_avals=tuple(out_avals),
            in_names=tuple(all_in_names),
            out_names=tuple(out_names),
            lowering_input_output_aliases=(),
            sim_require_finite=True,
            sim_require_nnan=True,
            nc=nc,
        )
        return tuple(outs)

    devices = jax.devices()[:n_cores]
    mesh = Mesh(np.asarray(devices), ("core",))
    spec = PartitionSpec("core")
    in_specs = (spec,) * (n_params + n_outs)
    out_specs = (spec,) * n_outs
    donate = tuple(range(n_params, n_params + n_outs))
    sharded = jax.jit(
        shard_map(_body, mesh=mesh, in_specs=in_specs, out_specs=out_specs,
                  check_rep=False),
        donate_argnums=donate, keep_unused=True)

    sh = NamedSharding(mesh, spec)
    concat_in = [
        jax.device_put(
            np.concatenate([np.asarray(in_maps[c][nm]) for c in range(n_cores)],
                           axis=0), sh)
        for nm in in_names]
    def put_zeros():
        return [jax.device_put(
                    np.zeros((n_cores * z.shape[0], *z.shape[1:]), z.dtype), sh)
                for z in zero_outs]

    out_arrs = sharded(*concat_in, *put_zeros())
    jax.block_until_ready(out_arrs)
    results = [
        {nm: np.asarray(out_arrs[i]).reshape(n_cores, *out_avals[i].shape)[c]
         for i, nm in enumerate(out_names)}
        for c in range(n_cores)]

    best = None
    for _ in range(bench_iters):
        zs = put_zeros()
        jax.block_until_ready(zs)
        t0 = time.perf_counter()
        o = sharded(*concat_in, *zs)
        jax.block_until_ready(o)
        dt = time.perf_counter() - t0
        best = dt if best is None or dt < best else best
    return results, best

